# revision 7
# baseline (speedup 1.0000x reference)
"""Trainium2 Bass kernel for BipolarMorphological2D (SMorph smooth-max).

Math
----
The reference computes, per (patch-sign i, kernel j):
    z_p  = log(max(+-x patch, 0.1)) + k_j[p]      (p over K*K*C = 288)
    y_ij = exp( sum_p z_p softmax_p(z_p) )
    out  = y11 - y12 - y21 + y22 + bias

Since exp(z_p) = a_p * E_p with a_p = max(+-x patch, 0.1), E_p = exp(k[p]):
    S0 = sum_p a_p E_p                      (softmax denominator)
    S1 = sum_p (a_p ln a_p) E_p + a_p F_p   (numerator; F = k * exp(k))
    y  = exp(S1 / S0),  1/S0 computed as exp(-ln S0) on the Scalar engine
Both S0 and S1 are matmuls over p=288, run on the TensorEngine as 3
PSUM-accumulated K=96 matmuls over shifted views of a "cropped" x tensor
(the 3 w-shifts stacked along partitions, the h-shift = free-dim offset).

Sharding: 8 cores = batch(4) x output-row-half(2). Each core computes
[O=64, 15*30=450] output from x[b, :, h0:h0+17, :].
"""

import sys

sys.path.insert(0, "/opt/trn_rl_repo")

from contextlib import ExitStack

import numpy as np

import bass_rust
import concourse.bass as bass
import concourse.mybir as mybir
import concourse.tile as tile
from concourse import bass_utils

F32 = mybir.dt.float32
F32R = mybir.dt.float32r
AFT = mybir.ActivationFunctionType
ALU = mybir.AluOpType

B, C, H, W, O = 4, 32, 32, 32, 64
KK = 3
HO = WO = H - KK + 1  # 30
HHALF = HO // 2  # 15 output rows per core
XROWS = HHALF + KK - 1  # 17 input rows per core
N = HHALF * WO  # 450 output pixels per core
PS = 3 * C  # 96 patch rows per h-shift group
CROP = XROWS * WO  # 510
INPUT_SHIFT = 0.1

# matmul dtype for the contraction: float32r = 1 cycle/row, float32 = 4
MM_DT = F32R


# tiles that feed the PE as f32r must be *produced* as f32r (the BIR
# verifier requires producer outputs pre-rounded to f32r)
def _r(ap):
    return ap


def split_excess_waits(nc):
    """This walrus build caps sync waits at 1/inst (2 for EventSemaphore).
    Tile's tail drain can carry more; move extras onto EventSemaphore
    carriers inserted right before the offender on the same engine."""
    ctr = 0
    for f in nc.m.functions:
        for b in f.blocks:
            new = []
            changed = False
            for inst in b.instructions:
                si = inst.sync_info
                cap = 2 if inst.opcode == "EventSemaphore" else 1
                if si is not None and len(si.on_wait) > cap:
                    waits = list(si.on_wait)
                    keep, rest = waits[:cap], waits[cap:]
                    while rest:
                        chunk, rest = rest[:2], rest[2:]
                        es = mybir.InstEventSemaphore(
                            name=f"wsplit_{ctr}", ins=[], outs=[]
                        )
                        ctr += 1
                        es.engine = inst.engine
                        es.sync_info = bass_rust.SyncInfo(on_wait=chunk, on_update=[])
                        new.append(es)
                    inst.sync_info = bass_rust.SyncInfo(
                        on_wait=keep, on_update=list(si.on_update)
                    )
                    changed = True
                new.append(inst)
            if changed:
                b.instructions = new
    return ctr


def build_nc():
    nc = bass.Bass("TRN2", target_bir_lowering=False, debug=False)
    x_ap = nc.dram_tensor("x", [C, XROWS * W], F32, kind="ExternalInput").ap()
    # k12: [96, 3*128], col = i*128 + jkern*64 + o (matmul lhsT layout)
    k12_ap = nc.dram_tensor("k12", [PS, 3 * 128], F32, kind="ExternalInput").ap()
    bias_ap = nc.dram_tensor("bias", [O, 1], F32, kind="ExternalInput").ap()
    y_ap = nc.dram_tensor("y", [O, N], F32, kind="ExternalOutput").ap()

    with tile.TileContext(nc) as tc, ExitStack() as ctx:
        pool = ctx.enter_context(tc.tile_pool(name="main", bufs=1))
        psum = ctx.enter_context(tc.tile_pool(name="psum", bufs=1, space="PSUM"))

        # ---- loads ----
        xb = pool.tile([C, XROWS * W], F32)
        nc.sync.dma_start(xb[:], x_ap)
        k12 = pool.tile([PS, 3 * 128], F32)
        nc.sync.dma_start(k12[:], k12_ap)
        biast = pool.tile([O, 1], F32)
        nc.sync.dma_start(biast[:], bias_ap)

        # ---- weight transforms: WE = exp(k), WF = k*exp(k), [96, 3*128] ----
        WE = pool.tile([PS, 3 * 128], MM_DT)
        nc.scalar.activation(WE[:], k12[:], AFT.Exp)
        WF = pool.tile([PS, 3 * 128], MM_DT)
        nc.vector.tensor_mul(WF[:], k12[:], WE[:])

        # ---- x crop [96, 510]: partition block j = w-shift, free = (h, wo) ----
        xc = pool.tile([PS, CROP], F32)
        xbv = xb[:].rearrange("p (h w) -> p h w", h=XROWS)
        xcv = xc[:].rearrange("p (h w) -> p h w", h=XROWS)
        for j in range(3):
            nc.sync.dma_start(xcv[32 * j : 32 * (j + 1)], xbv[:, :, j : j + WO])

        # ab = [a1 | a2] with a = max(+-x, 0.1);  Lab = ab * ln(ab)
        ab = pool.tile([PS, 2 * CROP], MM_DT)
        nc.vector.tensor_scalar_max(ab[:, 0:CROP], xc[:], INPUT_SHIFT)
        nc.vector.tensor_scalar(
            ab[:, CROP : 2 * CROP], xc[:], -1.0, INPUT_SHIFT,
            op0=ALU.mult, op1=ALU.max,
        )
        lnab = pool.tile([PS, 2 * CROP], F32)
        nc.scalar.activation(lnab[:], ab[:], AFT.Ln)
        Lab = pool.tile([PS, 2 * CROP], MM_DT)
        nc.gpsimd.tensor_mul(Lab[:], ab[:], lnab[:])

        # ---- matmuls ----
        # branch br: a view = ab[:, br*510 : ...], L view = Lab[:, br*510 : ...]
        # shift i rhs = view[:, i*30 : i*30+450] (contiguous in the crop)
        def shifted(t, br, i):
            off = br * CROP + i * WO
            return _r(t[:, off : off + N])

        # paired PSUM tiles: [128, 1024] = 2 banks; branch br at cols br*512
        s0p = psum.tile([128, 1024], F32)
        s1p = psum.tile([128, 1024], F32)
        for br in range(2):
            s0 = s0p[:, br * 512 : br * 512 + N]
            s1 = s1p[:, br * 512 : br * 512 + N]
            for i in range(3):
                nc.tensor.matmul(
                    s0,
                    lhsT=_r(WE[:, i * 128 : (i + 1) * 128]),
                    rhs=shifted(ab, br, i),
                    start=(i == 0),
                    stop=(i == 2),
                )
            for i in range(3):
                nc.tensor.matmul(
                    s1,
                    lhsT=_r(WE[:, i * 128 : (i + 1) * 128]),
                    rhs=shifted(Lab, br, i),
                    start=(i == 0),
                    stop=False,
                )
            for i in range(3):
                nc.tensor.matmul(
                    s1,
                    lhsT=_r(WF[:, i * 128 : (i + 1) * 128]),
                    rhs=shifted(ab, br, i),
                    start=False,
                    stop=(i == 2),
                )

        # ---- epilogue (branch pairs batched as [128, 2x450]) ----
        s0v = s0p[:].rearrange("p (u v) -> p u v", u=2)[:, :, 0:N]  # [128,2,450]
        s1v = s1p[:].rearrange("p (u v) -> p u v", u=2)[:, :, 0:N]

        u = pool.tile([128, 2 * N], F32)
        uv = u[:].rearrange("p (u v) -> p u v", u=2)
        nc.scalar.activation(uv, s0v, AFT.Ln)
        r = pool.tile([128, 2 * N], F32)
        nc.scalar.activation(r[:], u[:], AFT.Exp, scale=-1.0)  # r = 1/S0
        t = pool.tile([128, 2 * N], F32)
        tv = t[:].rearrange("p (u v) -> p u v", u=2)
        nc.vector.tensor_mul(tv, s1v, r[:].rearrange("p (u v) -> p u v", u=2))
        y = pool.tile([128, 2 * N], F32)
        nc.scalar.activation(y[:], t[:], AFT.Exp)  # [y_1 | y_2] stacks

        # ---- combine: out = (y11 - y21) - (y12 - y22) + bias ----
        # d split so the final op sees SBUF@base0 + PSUM@base64 (the DVE
        # rejects two SBUF operands at different base partitions)
        d_top = pool.tile([O, N], F32)
        nc.vector.tensor_sub(d_top[:], y[0:O, 0:N], y[0:O, N : 2 * N])
        d_bot = psum.tile([128, 512], F32)
        nc.vector.tensor_sub(
            d_bot[O : 2 * O, 0:N], y[O : 2 * O, 0:N], y[O : 2 * O, N : 2 * N]
        )
        out_sb = pool.tile([O, N], F32)
        # (d_top + bias) - d_bot
        nc.vector.scalar_tensor_tensor(
            out_sb[:], d_top[:], biast[:], d_bot[O : 2 * O, 0:N],
            op0=ALU.add, op1=ALU.subtract,
        )
        nc.sync.dma_start(y_ap, out_sb[:])

    split_excess_waits(nc)
    return nc


_nc_cache = None


def _get_nc():
    global _nc_cache
    if _nc_cache is None:
        _nc_cache = build_nc()
    return _nc_cache


def _host_inputs(x, k1, k2, bias):
    """Build the 8 per-core input maps (pure layout, no arithmetic)."""
    # k [3,3,C,O] -> p=(i*3+j)*C+c -> [3(i), 96, O]; stack k1,k2 in M chunks
    k1f = np.ascontiguousarray(k1, np.float32).reshape(3, PS, O)
    k2f = np.ascontiguousarray(k2, np.float32).reshape(3, PS, O)
    k12 = np.concatenate([k1f, k2f], axis=2)  # [3, 96, 128]
    k12_sb = np.ascontiguousarray(k12.transpose(1, 0, 2).reshape(PS, 3 * 128))
    bias_sb = np.ascontiguousarray(bias, np.float32).reshape(O, 1)

    in_maps = []
    for core in range(8):
        b, half = divmod(core, 2)
        h0 = half * HHALF
        xs = np.ascontiguousarray(x[b, :, h0 : h0 + XROWS, :], np.float32)
        in_maps.append(
            {"x": xs.reshape(C, XROWS * W), "k12": k12_sb, "bias": bias_sb}
        )
    return in_maps


def kernel(x, k1, k2, bias):
    nc = _get_nc()
    in_maps = _host_inputs(x, k1, k2, bias)
    res = bass_utils.run_bass_kernel_spmd(
        nc, in_maps, core_ids=list(range(8)), trace=False
    )
    out = np.empty((B, O, HO, WO), np.float32)
    for core in range(8):
        b, half = divmod(core, 2)
        h0 = half * HHALF
        out[b, :, h0 : h0 + HHALF, :] = res.results[core]["y"].reshape(O, HHALF, WO)
    return out


if __name__ == "__main__":
    rng = np.random.default_rng(0)
    x = rng.standard_normal((B, C, H, W), dtype=np.float32)
    k1 = ((rng.random((KK, KK, C, O)) - 0.5) * 0.16).astype(np.float32)
    k2 = ((rng.random((KK, KK, C, O)) - 0.5) * 0.16).astype(np.float32)
    bias = np.zeros((O,), np.float32)
    out = kernel(x, k1, k2, bias)
    print("kernel out:", out.shape, out.dtype, float(np.abs(out).max()))


# revision 10
# speedup vs baseline: 1.3993x; 1.3993x over previous
"""Trainium2 Bass kernel for BipolarMorphological2D (SMorph smooth-max).

Math
----
The reference computes, per (patch-sign i, kernel j):
    z_p  = log(max(+-x patch, 0.1)) + k_j[p]      (p over K*K*C = 288)
    y_ij = exp( sum_p z_p softmax_p(z_p) )
    out  = y11 - y12 - y21 + y22 + bias

Since exp(z_p) = a_p * E_p with a_p = max(+-x patch, 0.1), E_p = exp(k[p]):
    S0 = sum_p a_p E_p                      (softmax denominator)
    S1 = sum_p (a_p ln a_p) E_p + a_p F_p   (numerator; F = k * exp(k))
    y  = exp(S1 / S0),  1/S0 computed as exp(-ln S0) on the Scalar engine
Both S0 and S1 are matmuls over p=288, run on the TensorEngine as 3
PSUM-accumulated K=96 matmuls over shifted views of a "cropped" x tensor
(the 3 w-shifts stacked along partitions, the h-shift = free-dim offset).

Sharding: 8 cores = batch(4) x output-row-half(2). Each core computes
[O=64, 15*30=450] output from x[b, :, h0:h0+17, :].
"""

import sys

sys.path.insert(0, "/opt/trn_rl_repo")

from contextlib import ExitStack

import numpy as np

import bass_rust
import concourse.bass as bass
import concourse.mybir as mybir
import concourse.tile as tile
from concourse import bass_utils

F32 = mybir.dt.float32
F32R = mybir.dt.float32r
AFT = mybir.ActivationFunctionType
ALU = mybir.AluOpType

B, C, H, W, O = 4, 32, 32, 32, 64
KK = 3
HO = WO = H - KK + 1  # 30
HHALF = HO // 2  # 15 output rows per core
XROWS = HHALF + KK - 1  # 17 input rows per core
N = HHALF * WO  # 450 output pixels per core
PS = 3 * C  # 96 patch rows per h-shift group
CROP = XROWS * WO  # 510
INPUT_SHIFT = 0.1

# matmul dtype for the contraction: float32r = 1 cycle/row, float32 = 4
MM_DT = F32R
WARMUP_MMS = 5


# tiles that feed the PE as f32r must be *produced* as f32r (the BIR
# verifier requires producer outputs pre-rounded to f32r)
def _r(ap):
    return ap


def split_excess_waits(nc):
    """This walrus build caps sync waits at 1/inst (2 for EventSemaphore).
    Tile's tail drain can carry more; move extras onto EventSemaphore
    carriers inserted right before the offender on the same engine."""
    ctr = 0
    for f in nc.m.functions:
        for b in f.blocks:
            new = []
            changed = False
            for inst in b.instructions:
                si = inst.sync_info
                cap = 2 if inst.opcode == "EventSemaphore" else 1
                if si is not None and len(si.on_wait) > cap:
                    waits = list(si.on_wait)
                    keep, rest = waits[:cap], waits[cap:]
                    while rest:
                        chunk, rest = rest[:2], rest[2:]
                        es = mybir.InstEventSemaphore(
                            name=f"wsplit_{ctr}", ins=[], outs=[]
                        )
                        ctr += 1
                        es.engine = inst.engine
                        es.sync_info = bass_rust.SyncInfo(on_wait=chunk, on_update=[])
                        new.append(es)
                    inst.sync_info = bass_rust.SyncInfo(
                        on_wait=keep, on_update=list(si.on_update)
                    )
                    changed = True
                new.append(inst)
            if changed:
                b.instructions = new
    return ctr


def build_nc():
    nc = bass.Bass("TRN2", target_bir_lowering=False, debug=False)
    # x declared f32r so plain (non-casting) DMAs feed the f32r matmul path
    x_ap = nc.dram_tensor("x", [C, XROWS * W], MM_DT, kind="ExternalInput").ap()
    # k12: [96, 3*128], col = i*128 + jkern*64 + o (matmul lhsT layout)
    k12_ap = nc.dram_tensor("k12", [PS, 3 * 128], MM_DT, kind="ExternalInput").ap()
    bias_ap = nc.dram_tensor("bias", [O, 1], F32, kind="ExternalInput").ap()
    y_ap = nc.dram_tensor("y", [O, N], F32, kind="ExternalOutput").ap()

    xv = x_ap.rearrange("c (h w) -> c h w", h=XROWS)

    with tile.TileContext(nc) as tc, ExitStack() as ctx:
        pool = ctx.enter_context(tc.tile_pool(name="main", bufs=1))
        psum = ctx.enter_context(tc.tile_pool(name="psum", bufs=1, space="PSUM"))

        # ---- PE warm-up: keep the PE busy during the DMA/prep phase so the
        # HAM clock is at full rate when the real matmuls arrive ----
        wsrc = pool.tile([128, 512], F32)
        nc.gpsimd.memset(wsrc[:], 1.0)
        warm_ps = psum.tile([128, 512], F32)
        for w in range(WARMUP_MMS):
            nc.tensor.matmul(
                warm_ps[:, 0:128], lhsT=wsrc[:, 0:128], rhs=wsrc[:, 0:128],
                start=True, stop=True,
            )

        # ---- input loads (SP + ACT HWDGE rings in parallel) ----
        xc = pool.tile([PS, CROP], MM_DT)
        xcv = xc[:].rearrange("p (h w) -> p h w", h=XROWS)
        for j in range(3):
            eng = nc.sync if j != 1 else nc.scalar
            eng.dma_start(xcv[32 * j : 32 * (j + 1)], xv[:, :, j : j + WO])
        k12 = pool.tile([PS, 3 * 128], MM_DT)
        nc.scalar.dma_start(k12[:], k12_ap)
        biast = pool.tile([O, 1], F32)
        nc.scalar.dma_start(biast[:], bias_ap)

        # ---- weight transforms: WE = exp(k), WF = k*exp(k), [96, 3*128] ----
        WE = pool.tile([PS, 3 * 128], MM_DT)
        nc.scalar.activation(WE[:], k12[:], AFT.Exp)
        WF = pool.tile([PS, 3 * 128], MM_DT)
        nc.vector.tensor_mul(WF[:], k12[:], WE[:])

        # a1 = max(x, 0.1); a2 = max(-x, 0.1)
        a1 = pool.tile([PS, CROP], MM_DT)
        nc.vector.tensor_scalar_max(a1[:], xc[:], INPUT_SHIFT)
        a2 = pool.tile([PS, CROP], MM_DT)
        nc.vector.tensor_scalar(
            a2[:], xc[:], -1.0, INPUT_SHIFT, op0=ALU.mult, op1=ALU.max
        )

        # L = a * ln a  (branch 1 on DVE, branch 2 on GPSIMD for overlap)
        ln1 = pool.tile([PS, CROP], F32)
        nc.scalar.activation(ln1[:], a1[:].bitcast(F32), AFT.Ln)
        ln2 = pool.tile([PS, CROP], F32)
        nc.scalar.activation(ln2[:], a2[:].bitcast(F32), AFT.Ln)
        L1 = pool.tile([PS, CROP], MM_DT)
        nc.vector.tensor_mul(L1[:], a1[:].bitcast(F32), ln1[:])
        L2 = pool.tile([PS, CROP], MM_DT)
        nc.gpsimd.tensor_mul(L2[:], a2[:].bitcast(F32), ln2[:])

        # ---- matmuls: 3 PSUM-accumulated K=96 matmuls per product ----
        def shifted(t, i):
            return t[:, i * WO : i * WO + N]

        s0 = []
        s1 = []
        for br, (a_t, l_t) in enumerate(((a1, L1), (a2, L2))):
            s0_t = psum.tile([128, N], F32, tag=f"s0_{br}", name=f"s0_{br}")
            for i in range(3):
                nc.tensor.matmul(
                    s0_t[:],
                    lhsT=WE[:, i * 128 : (i + 1) * 128],
                    rhs=shifted(a_t, i),
                    start=(i == 0),
                    stop=(i == 2),
                )
            s1_t = psum.tile([128, N], F32, tag=f"s1_{br}", name=f"s1_{br}")
            for i in range(3):
                nc.tensor.matmul(
                    s1_t[:],
                    lhsT=WE[:, i * 128 : (i + 1) * 128],
                    rhs=shifted(l_t, i),
                    start=(i == 0),
                    stop=False,
                )
            for i in range(3):
                nc.tensor.matmul(
                    s1_t[:],
                    lhsT=WF[:, i * 128 : (i + 1) * 128],
                    rhs=shifted(a_t, i),
                    start=False,
                    stop=(i == 2),
                )
            s0.append(s0_t)
            s1.append(s1_t)

        # ---- epilogue per branch: y = exp(S1 * exp(-ln S0)) ----
        us, rs, ts_, ys = [], [], [], []
        for br in range(2):
            u_t = pool.tile([128, N], F32, tag=f"u_{br}", name=f"u_{br}")
            nc.scalar.activation(u_t[:], s0[br][:], AFT.Ln)
            r_t = pool.tile([128, N], F32, tag=f"r_{br}", name=f"r_{br}")
            nc.scalar.activation(r_t[:], u_t[:], AFT.Exp, scale=-1.0)
            us.append(u_t)
            rs.append(r_t)
        for br in range(2):
            t_t = pool.tile([128, N], F32, tag=f"t_{br}", name=f"t_{br}")
            nc.vector.tensor_mul(t_t[:], s1[br][:], rs[br][:])
            ts_.append(t_t)
        for br in range(2):
            y_t = pool.tile([128, N], F32, tag=f"y_{br}", name=f"y_{br}")
            nc.scalar.activation(y_t[:], ts_[br][:], AFT.Exp)
            ys.append(y_t)

        # ---- combine: out = (y11 + bias - y21) - (y12 - y22) ----
        y1, y2 = ys
        tmp = pool.tile([O, N], F32)
        nc.vector.scalar_tensor_tensor(
            tmp[:], y1[0:O, :], biast[:], y2[0:O, :],
            op0=ALU.add, op1=ALU.subtract,
        )
        q = psum.tile([128, 512], F32)
        nc.vector.tensor_sub(q[O : 2 * O, 0:N], y1[O : 2 * O, :], y2[O : 2 * O, :])
        out_sb = pool.tile([O, N], F32)
        nc.vector.tensor_sub(out_sb[:], tmp[:], q[O : 2 * O, 0:N])
        nc.sync.dma_start(y_ap, out_sb[:])

    split_excess_waits(nc)
    return nc


_nc_cache = None


def _get_nc():
    global _nc_cache
    if _nc_cache is None:
        _nc_cache = build_nc()
    return _nc_cache


def _host_inputs(x, k1, k2, bias):
    """Build the 8 per-core input maps (pure layout, no arithmetic)."""
    # k [3,3,C,O] -> p=(i*3+j)*C+c -> [3(i), 96, O]; stack k1,k2 in M chunks
    k1f = np.ascontiguousarray(k1, np.float32).reshape(3, PS, O)
    k2f = np.ascontiguousarray(k2, np.float32).reshape(3, PS, O)
    k12 = np.concatenate([k1f, k2f], axis=2)  # [3, 96, 128]
    k12_sb = np.ascontiguousarray(k12.transpose(1, 0, 2).reshape(PS, 3 * 128))
    bias_sb = np.ascontiguousarray(bias, np.float32).reshape(O, 1)

    in_maps = []
    for core in range(8):
        b, half = divmod(core, 2)
        h0 = half * HHALF
        xs = np.ascontiguousarray(x[b, :, h0 : h0 + XROWS, :], np.float32)
        in_maps.append(
            {"x": xs.reshape(C, XROWS * W), "k12": k12_sb, "bias": bias_sb}
        )
    return in_maps


def kernel(x, k1, k2, bias):
    nc = _get_nc()
    in_maps = _host_inputs(x, k1, k2, bias)
    res = bass_utils.run_bass_kernel_spmd(
        nc, in_maps, core_ids=list(range(8)), trace=False
    )
    out = np.empty((B, O, HO, WO), np.float32)
    for core in range(8):
        b, half = divmod(core, 2)
        h0 = half * HHALF
        out[b, :, h0 : h0 + HHALF, :] = res.results[core]["y"].reshape(O, HHALF, WO)
    return out


if __name__ == "__main__":
    rng = np.random.default_rng(0)
    x = rng.standard_normal((B, C, H, W), dtype=np.float32)
    k1 = ((rng.random((KK, KK, C, O)) - 0.5) * 0.16).astype(np.float32)
    k2 = ((rng.random((KK, KK, C, O)) - 0.5) * 0.16).astype(np.float32)
    bias = np.zeros((O,), np.float32)
    out = kernel(x, k1, k2, bias)
    print("kernel out:", out.shape, out.dtype, float(np.abs(out).max()))


# revision 22
# speedup vs baseline: 1.5320x; 1.0949x over previous
"""Trainium2 Bass kernel for BipolarMorphological2D (SMorph smooth-max).

Math
----
The reference computes, per (patch-sign i, kernel j):
    z_p  = log(max(+-x patch, 0.1)) + k_j[p]      (p over K*K*C = 288)
    y_ij = exp( sum_p z_p softmax_p(z_p) )
    out  = y11 - y12 - y21 + y22 + bias

Since exp(z_p) = a_p * E_p with a_p = max(+-x patch, 0.1), E_p = exp(k[p]):
    S0 = sum_p a_p E_p                      (softmax denominator)
    S1 = sum_p (a_p ln a_p) E_p + a_p F_p   (numerator; F = k * exp(k))
    y  = exp(S1 / S0),  1/S0 computed as exp(-ln S0) on the Scalar engine
Both S0 and S1 are matmuls over p=288, run on the TensorEngine as 3
PSUM-accumulated K=96 matmuls over shifted views of a "cropped" x tensor
(the 3 w-shifts stacked along partitions, the h-shift = free-dim offset).

Sharding: 8 cores = batch(4) x output-row-half(2). Each core computes
[O=64, 15*30=450] output from x[b, :, h0:h0+17, :].
"""

import sys

sys.path.insert(0, "/opt/trn_rl_repo")

import dataclasses
from contextlib import ExitStack

import numpy as np

import bass_rust
import concourse.bass as bass
import concourse.mybir as mybir
import concourse.tile as tile
from concourse import bass_utils

F32 = mybir.dt.float32
F32R = mybir.dt.float32r
AFT = mybir.ActivationFunctionType
ALU = mybir.AluOpType

B, C, H, W, O = 4, 32, 32, 32, 64
KK = 3
HO = WO = H - KK + 1  # 30
HHALF = HO // 2  # 15 output rows per core
XROWS = HHALF + KK - 1  # 17 input rows per core
N = HHALF * WO  # 450 output pixels per core
PS = 3 * C  # 96 patch rows per h-shift group
CROP = XROWS * WO  # 510
XPAD = XROWS * W + 4  # padded x row length (548)
REPW = XROWS * W + 2  # replicated-row width (546)
INPUT_SHIFT = 0.1

# matmul dtype for the contraction: float32r = 1 cycle/row, float32 = 4
MM_DT = F32R
WARMUP_MMS = 5


# tiles that feed the PE as f32r must be *produced* as f32r (the BIR
# verifier requires producer outputs pre-rounded to f32r)
def _r(ap):
    return ap


def split_excess_waits(nc):
    """This walrus build caps sync waits at 1/inst (2 for EventSemaphore).
    Tile's tail drain can carry more; move extras onto EventSemaphore
    carriers inserted right before the offender on the same engine."""
    ctr = 0
    for f in nc.m.functions:
        for b in f.blocks:
            new = []
            changed = False
            for inst in b.instructions:
                si = inst.sync_info
                cap = 2 if inst.opcode == "EventSemaphore" else 1
                if si is not None and len(si.on_wait) > cap:
                    waits = list(si.on_wait)
                    keep, rest = waits[:cap], waits[cap:]
                    while rest:
                        chunk, rest = rest[:2], rest[2:]
                        es = mybir.InstEventSemaphore(
                            name=f"wsplit_{ctr}", ins=[], outs=[]
                        )
                        ctr += 1
                        es.engine = inst.engine
                        es.sync_info = bass_rust.SyncInfo(on_wait=chunk, on_update=[])
                        new.append(es)
                    inst.sync_info = bass_rust.SyncInfo(
                        on_wait=keep, on_update=list(si.on_update)
                    )
                    changed = True
                new.append(inst)
            if changed:
                b.instructions = new
    return ctr


def _chain(insts, reason):
    """Pin scheduling order on one engine: each inst depends on the prior."""
    for prev, cur in zip(insts, insts[1:]):
        if prev is not None and cur is not None:
            tile.add_dep_helper(cur.ins, prev.ins, sync=False, reason=reason)


def build_nc():
    nc = bass.Bass("TRN2", target_bir_lowering=False, debug=False)
    # x: host-replicated [96, REPW]: row j*32+c = x[c, j:j+REPW] (w-shift j
    # baked in by the host layout); f32r so plain DMAs feed the f32r matmuls
    x_ap = nc.dram_tensor("x", [PS, REPW], MM_DT, kind="ExternalInput").ap()
    # k12: [96, 3*128], col = i*128 + jkern*64 + o (matmul lhsT layout)
    k12_ap = nc.dram_tensor("k12", [PS, 3 * 128], MM_DT, kind="ExternalInput").ap()
    bias_ap = nc.dram_tensor("bias", [O, 1], F32, kind="ExternalInput").ap()
    signs_ap = nc.dram_tensor("signs", [128, 2 * O], F32, kind="ExternalInput").ap()
    y_ap = nc.dram_tensor("y", [O, N], F32, kind="ExternalOutput").ap()

    with tile.TileContext(nc) as tc, ExitStack() as ctx:
        pool = ctx.enter_context(tc.tile_pool(name="main", bufs=1))
        psum = ctx.enter_context(tc.tile_pool(name="psum", bufs=1, space="PSUM"))

        # ---- PE warm-up: keep the PE busy during the DMA/prep phase so the
        # HAM clock is at full rate when the real matmuls arrive ----
        wsrc = pool.tile([128, 512], F32)
        nc.gpsimd.memset(wsrc[:], 1.0)
        # dummy exp so the ACT_TABLE_LOAD (~2.7us on HW) overlaps the DMA
        # phase instead of stalling the first real activation
        actwarm = pool.tile([128, 1], F32)
        aw_inst = nc.scalar.activation(actwarm[:], wsrc[:, 0:1], AFT.Exp)
        warm_ps = psum.tile([128, 512], F32)
        for w in range(WARMUP_MMS):
            nc.tensor.matmul(
                warm_ps[:, 0:128], lhsT=wsrc[:, 0:128], rhs=wsrc[:, 0:128],
                start=True, stop=True,
            )

        # ---- input loads, one DMA each (HWDGE desc-gen serializes; x
        # first: it gates the longest chain) ----
        # xrep [96, REPW]: one plain DMA of the host-replicated layout
        xrep = pool.tile([PS, REPW], MM_DT)
        nc.sync.dma_start(xrep[:], x_ap)
        k12 = pool.tile([PS, 3 * 128], MM_DT)
        nc.sync.dma_start(k12[:], k12_ap)
        biast = pool.tile([O, 1], F32)
        nc.sync.dma_start(biast[:], bias_ap)
        signst = pool.tile([128, 2 * O], F32)
        nc.sync.dma_start(signst[:], signs_ap)

        # ---- weight transforms: WE = exp(k), WF = k*exp(k), [96, 3*128] ----
        WE = pool.tile([PS, 3 * 128], MM_DT)
        we_inst = nc.scalar.activation(WE[:], k12[:], AFT.Exp)

        # ---- crops + clamps: a = max(+-x, 0.1) into [96, 510] (contig) ----
        # crop view of xrep: rows h=0..16, cols w=0..29 (w>=30 never used)
        xcrop = (
            xrep[:, 0 : XROWS * W]
            .rearrange("p (h w) -> p h w", w=W)[:, :, 0:WO]
        )
        a1 = pool.tile([PS, CROP], MM_DT)
        a1v = a1[:].rearrange("p (h w) -> p h w", h=XROWS)
        a1_inst = nc.vector.tensor_scalar_max(a1v, xcrop, INPUT_SHIFT)
        a2 = pool.tile([PS, CROP], MM_DT)
        a2v = a2[:].rearrange("p (h w) -> p h w", h=XROWS)
        a2_inst = nc.vector.tensor_scalar(
            a2v, xcrop, -1.0, INPUT_SHIFT, op0=ALU.mult, op1=ALU.max
        )

        WF = pool.tile([PS, 3 * 128], MM_DT)
        wf_inst = nc.vector.tensor_mul(WF[:], k12[:], WE[:])

        # L = a * ln a
        ln1 = pool.tile([PS, CROP], F32)
        ln1_inst = nc.scalar.activation(ln1[:], a1[:].bitcast(F32), AFT.Ln)
        ln2 = pool.tile([PS, CROP], F32)
        ln2_inst = nc.scalar.activation(ln2[:], a2[:].bitcast(F32), AFT.Ln)
        L1 = pool.tile([PS, CROP], MM_DT)
        l1_inst = nc.vector.tensor_mul(L1[:], a1[:].bitcast(F32), ln1[:])
        L2 = pool.tile([PS, CROP], MM_DT)
        l2_inst = nc.vector.tensor_mul(L2[:], a2[:].bitcast(F32), ln2[:])

        # ---- matmuls: 3 PSUM-accumulated K=96 matmuls per product.
        # S0 for both branches first so the ACT ln/exp of 1/S0 runs under
        # the S1 matmuls. ----
        def shifted(t, i):
            return t[:, i * WO : i * WO + N]

        s0p = psum.tile([128, 1024], F32)
        mms = []
        s1 = []
        for br, a_t in enumerate((a1, a2)):
            s0_sl = s0p[:, br * 512 : br * 512 + N]
            for i in range(3):
                mms.append(nc.tensor.matmul(
                    s0_sl,
                    lhsT=WE[:, i * 128 : (i + 1) * 128],
                    rhs=shifted(a_t, i),
                    start=(i == 0),
                    stop=(i == 2),
                ))
        for br, (a_t, l_t) in enumerate(((a1, L1), (a2, L2))):
            s1_t = psum.tile([128, N], F32, tag=f"s1_{br}", name=f"s1_{br}")
            for i in range(3):
                mms.append(nc.tensor.matmul(
                    s1_t[:],
                    lhsT=WE[:, i * 128 : (i + 1) * 128],
                    rhs=shifted(l_t, i),
                    start=(i == 0),
                    stop=False,
                ))
            for i in range(3):
                mms.append(nc.tensor.matmul(
                    s1_t[:],
                    lhsT=WF[:, i * 128 : (i + 1) * 128],
                    rhs=shifted(a_t, i),
                    start=False,
                    stop=(i == 2),
                ))
            s1.append(s1_t)
        _chain(mms, "PE: S0 groups then S1 groups in order")

        # ---- epilogue: y = exp(S1 * exp(-ln S0)); u/r batched over both
        # branches (one ACT op pair), t/y per branch ----
        ur_insts = []
        rts = []
        for br in range(2):
            u_t = pool.tile([128, N], F32, name=f"u_{br}")
            ur_insts.append(
                nc.scalar.activation(u_t[:], s0p[:, br * 512 : br * 512 + N],
                                     AFT.Ln))
            r_t = pool.tile([128, N], F32, name=f"r_{br}")
            ur_insts.append(
                nc.scalar.activation(r_t[:], u_t[:], AFT.Exp, scale=-1.0))
            rts.append(r_t)
        ys = []
        t_insts, y_insts = [], []
        for br in range(2):
            t_t = pool.tile([128, N], F32, name=f"t_{br}")
            t_insts.append(nc.vector.tensor_mul(t_t[:], s1[br][:], rts[br][:]))
            y_t = pool.tile([128, N], F32, name=f"y_{br}")
            y_insts.append(nc.scalar.activation(y_t[:], t_t[:], AFT.Exp))
            ys.append(y_t)

        # ---- combine on the PE: out_ps = signs_pos.T @ y1 + signs_neg.T @ y2
        # (signs = [+I;-I] and [-I;+I]), then bias-add + PSUM->SBUF on ACT ----
        y1, y2 = ys
        out_ps = psum.tile([O, 512], F32)
        mms.append(nc.tensor.matmul(
            out_ps[:, 0:N], lhsT=signst[:, 0:O], rhs=y1[:],
            start=True, stop=False,
        ))
        mms.append(nc.tensor.matmul(
            out_ps[:, 0:N], lhsT=signst[:, O : 2 * O], rhs=y2[:],
            start=False, stop=True,
        ))
        _chain(mms[-3:], "PE: combine after S1")
        out_sb = pool.tile([O, N], F32)
        bc_inst = nc.scalar.add(out_sb[:], out_ps[:, 0:N], biast[:])
        nc.sync.dma_start(y_ap, out_sb[:])

        # pin per-engine scheduling order along the dataflow
        _chain([aw_inst, we_inst, ln1_inst, ln2_inst] + ur_insts + y_insts
               + [bc_inst], "ACT order")
        _chain([a1_inst, a2_inst, wf_inst, l1_inst, l2_inst] + t_insts,
               "DVE order")

    split_excess_waits(nc)
    return nc


_nc_cache = None


def _get_nc():
    global _nc_cache
    if _nc_cache is None:
        _nc_cache = build_nc()
    return _nc_cache


def _host_inputs(x, k1, k2, bias):
    """Build the 8 per-core input maps (pure layout, no arithmetic)."""
    # k [3,3,C,O] -> p=(i*3+j)*C+c -> [3(i), 96, O]; stack k1,k2 in M chunks
    k1f = np.ascontiguousarray(k1, np.float32).reshape(3, PS, O)
    k2f = np.ascontiguousarray(k2, np.float32).reshape(3, PS, O)
    k12 = np.concatenate([k1f, k2f], axis=2)  # [3, 96, 128]
    k12_sb = np.ascontiguousarray(k12.transpose(1, 0, 2).reshape(PS, 3 * 128))
    bias_sb = np.ascontiguousarray(bias, np.float32).reshape(O, 1)
    eye = np.eye(O, dtype=np.float32)
    signs = np.concatenate(
        [np.concatenate([eye, -eye], axis=0), np.concatenate([-eye, eye], axis=0)],
        axis=1,
    )  # [128, 128]: [:, 0:64] = [+I;-I] for y1, [:, 64:128] = [-I;+I] for y2

    in_maps = []
    for core in range(8):
        b, half = divmod(core, 2)
        h0 = half * HHALF
        xrow = np.ones((C, XPAD), np.float32)
        xrow[:, 0 : XROWS * W] = x[b, :, h0 : h0 + XROWS, :].reshape(
            C, XROWS * W
        )
        xs = np.empty((PS, REPW), np.float32)
        for j in range(3):
            xs[j * C : (j + 1) * C, :] = xrow[:, j : j + REPW]
        in_maps.append(
            {"x": xs, "k12": k12_sb, "bias": bias_sb, "signs": signs}
        )
    return in_maps


def kernel(x, k1, k2, bias):
    nc = _get_nc()
    in_maps = _host_inputs(x, k1, k2, bias)
    res = bass_utils.run_bass_kernel_spmd(
        nc, in_maps, core_ids=list(range(8)), trace=False
    )
    out = np.empty((B, O, HO, WO), np.float32)
    for core in range(8):
        b, half = divmod(core, 2)
        h0 = half * HHALF
        out[b, :, h0 : h0 + HHALF, :] = res.results[core]["y"].reshape(O, HHALF, WO)
    return out


if __name__ == "__main__":
    rng = np.random.default_rng(0)
    x = rng.standard_normal((B, C, H, W), dtype=np.float32)
    k1 = ((rng.random((KK, KK, C, O)) - 0.5) * 0.16).astype(np.float32)
    k2 = ((rng.random((KK, KK, C, O)) - 0.5) * 0.16).astype(np.float32)
    bias = np.zeros((O,), np.float32)
    out = kernel(x, k1, k2, bias)
    print("kernel out:", out.shape, out.dtype, float(np.abs(out).max()))


# revision 26
# speedup vs baseline: 1.6146x; 1.0539x over previous
"""Trainium2 Bass kernel for BipolarMorphological2D (SMorph smooth-max).

Math
----
The reference computes, per (patch-sign i, kernel j):
    z_p  = log(max(+-x patch, 0.1)) + k_j[p]      (p over K*K*C = 288)
    y_ij = exp( sum_p z_p softmax_p(z_p) )
    out  = y11 - y12 - y21 + y22 + bias

Since exp(z_p) = a_p * E_p with a_p = max(+-x patch, 0.1), E_p = exp(k[p]):
    S0 = sum_p a_p E_p                      (softmax denominator)
    S1 = sum_p (a_p ln a_p) E_p + a_p F_p   (numerator; F = k * exp(k))
    y  = exp(S1 / S0),  1/S0 computed as exp(-ln S0) on the Scalar engine
Both S0 and S1 are matmuls over p=288, run on the TensorEngine as 3
PSUM-accumulated K=96 matmuls over shifted views of a "cropped" x tensor
(the 3 w-shifts stacked along partitions, the h-shift = free-dim offset).

Sharding: 8 cores = batch(4) x output-row-half(2). Each core computes
[O=64, 15*30=450] output from x[b, :, h0:h0+17, :].
"""

import sys

sys.path.insert(0, "/opt/trn_rl_repo")

import dataclasses
from contextlib import ExitStack

import numpy as np

import bass_rust
import concourse.bass as bass
import concourse.mybir as mybir
import concourse.tile as tile
from concourse import bass_utils

F32 = mybir.dt.float32
F32R = mybir.dt.float32r
AFT = mybir.ActivationFunctionType
ALU = mybir.AluOpType

B, C, H, W, O = 4, 32, 32, 32, 64
KK = 3
HO = WO = H - KK + 1  # 30
HHALF = HO // 2  # 15 output rows per core
XROWS = HHALF + KK - 1  # 17 input rows per core
N = HHALF * WO  # 450 output pixels per core
PS = 3 * C  # 96 patch rows per h-shift group
CROP = XROWS * WO  # 510
XPAD = XROWS * W + 4  # padded x row length (548)
REPW = XROWS * W + 2  # replicated-row width (546)
INPUT_SHIFT = 0.1

# matmul dtype for the contraction: float32r = 1 cycle/row, float32 = 4
MM_DT = F32R
WARMUP_MMS = 5


# tiles that feed the PE as f32r must be *produced* as f32r (the BIR
# verifier requires producer outputs pre-rounded to f32r)
def _r(ap):
    return ap


def split_excess_waits(nc):
    """This walrus build caps sync waits at 1/inst (2 for EventSemaphore).
    Tile's tail drain can carry more; move extras onto EventSemaphore
    carriers inserted right before the offender on the same engine."""
    ctr = 0
    for f in nc.m.functions:
        for b in f.blocks:
            new = []
            changed = False
            for inst in b.instructions:
                si = inst.sync_info
                cap = 2 if inst.opcode == "EventSemaphore" else 1
                if si is not None and len(si.on_wait) > cap:
                    waits = list(si.on_wait)
                    keep, rest = waits[:cap], waits[cap:]
                    while rest:
                        chunk, rest = rest[:2], rest[2:]
                        es = mybir.InstEventSemaphore(
                            name=f"wsplit_{ctr}", ins=[], outs=[]
                        )
                        ctr += 1
                        es.engine = inst.engine
                        es.sync_info = bass_rust.SyncInfo(on_wait=chunk, on_update=[])
                        new.append(es)
                    inst.sync_info = bass_rust.SyncInfo(
                        on_wait=keep, on_update=list(si.on_update)
                    )
                    changed = True
                new.append(inst)
            if changed:
                b.instructions = new
    return ctr


def _chain(insts, reason):
    """Pin scheduling order on one engine: each inst depends on the prior."""
    for prev, cur in zip(insts, insts[1:]):
        if prev is not None and cur is not None:
            tile.add_dep_helper(cur.ins, prev.ins, sync=False, reason=reason)


def build_nc():
    nc = bass.Bass("TRN2", target_bir_lowering=False, debug=False)
    # x: host-replicated [96, REPW]: row j*32+c = x[c, j:j+REPW] (w-shift j
    # baked in by the host layout); f32r so plain DMAs feed the f32r matmuls
    x_ap = nc.dram_tensor("x", [PS, REPW], MM_DT, kind="ExternalInput").ap()
    # k12: [96, 3*128], col = i*128 + jkern*64 + o (matmul lhsT layout)
    k12_ap = nc.dram_tensor("k12", [PS, 3 * 128], MM_DT, kind="ExternalInput").ap()
    bias_ap = nc.dram_tensor("bias", [O, 1], F32, kind="ExternalInput").ap()
    signs_ap = nc.dram_tensor("signs", [128, 2 * O], F32, kind="ExternalInput").ap()
    y_ap = nc.dram_tensor("y", [O, N], F32, kind="ExternalOutput").ap()

    with tile.TileContext(nc) as tc, ExitStack() as ctx:
        pool = ctx.enter_context(tc.tile_pool(name="main", bufs=1))
        psum = ctx.enter_context(tc.tile_pool(name="psum", bufs=1, space="PSUM"))

        # ---- PE warm-up: keep the PE busy during the DMA/prep phase so the
        # HAM clock is at full rate when the real matmuls arrive ----
        wsrc = pool.tile([128, 512], F32)
        nc.gpsimd.memset(wsrc[:], 1.0)
        # dummy exp so the ACT_TABLE_LOAD (~2.7us on HW) overlaps the DMA
        # phase instead of stalling the first real activation
        actwarm = pool.tile([128, 1], F32)
        aw_inst = nc.scalar.activation(actwarm[:], wsrc[:, 0:1], AFT.Exp)
        warm_ps = psum.tile([128, 512], F32, tag="warm_ps")
        for w in range(WARMUP_MMS):
            nc.tensor.matmul(
                warm_ps[:, 0:128], lhsT=wsrc[:, 0:128], rhs=wsrc[:, 0:128],
                start=True, stop=True,
            )

        # ---- input loads, one DMA each (HWDGE desc-gen serializes; x
        # first: it gates the longest chain) ----
        # k12 first: it gates WE -> all matmuls and the whole ACT chain
        k12 = pool.tile([PS, 3 * 128], MM_DT)
        nc.sync.dma_start(k12[:], k12_ap)
        # xrep [96, REPW]: one plain DMA of the host-replicated layout
        xrep = pool.tile([PS, REPW], MM_DT)
        nc.sync.dma_start(xrep[:], x_ap)
        biast = pool.tile([O, 1], F32)
        nc.sync.dma_start(biast[:], bias_ap)
        signst = pool.tile([128, 2 * O], F32)
        nc.sync.dma_start(signst[:], signs_ap)

        # ---- weight transforms: WE = exp(k), WF = k*exp(k), [96, 3*128] ----
        WE = pool.tile([PS, 3 * 128], MM_DT)
        we_inst = nc.scalar.activation(WE[:], k12[:], AFT.Exp)

        # ---- crops + clamps: a = max(+-x, 0.1) into [96, 510] (contig) ----
        # crop view of xrep: rows h=0..16, cols w=0..29 (w>=30 never used)
        xcrop = (
            xrep[:, 0 : XROWS * W]
            .rearrange("p (h w) -> p h w", w=W)[:, :, 0:WO]
        )
        a1 = pool.tile([PS, CROP], MM_DT)
        a1v = a1[:].rearrange("p (h w) -> p h w", h=XROWS)
        a1_inst = nc.vector.tensor_scalar_max(a1v, xcrop, INPUT_SHIFT)
        a2 = pool.tile([PS, CROP], MM_DT)
        a2v = a2[:].rearrange("p (h w) -> p h w", h=XROWS)
        a2_inst = nc.vector.tensor_scalar(
            a2v, xcrop, -1.0, INPUT_SHIFT, op0=ALU.mult, op1=ALU.max
        )

        WF = pool.tile([PS, 3 * 128], MM_DT)
        wf_inst = nc.vector.tensor_mul(WF[:], k12[:], WE[:])

        # L = a * ln a
        ln1 = pool.tile([PS, CROP], F32)
        ln1_inst = nc.scalar.activation(ln1[:], a1[:].bitcast(F32), AFT.Ln)
        ln2 = pool.tile([PS, CROP], F32)
        ln2_inst = nc.scalar.activation(ln2[:], a2[:].bitcast(F32), AFT.Ln)
        L1 = pool.tile([PS, CROP], MM_DT)
        l1_inst = nc.vector.tensor_mul(L1[:], a1[:].bitcast(F32), ln1[:])
        L2 = pool.tile([PS, CROP], MM_DT)
        l2_inst = nc.vector.tensor_mul(L2[:], a2[:].bitcast(F32), ln2[:])

        # ---- matmuls: 3 PSUM-accumulated K=96 matmuls per product.
        # S0 for both branches first so the ACT ln/exp of 1/S0 runs under
        # the S1 matmuls. ----
        def shifted(t, i):
            return t[:, i * WO : i * WO + N]

        s0p = psum.tile([128, 1024], F32)
        mms = []
        s1 = []
        for br, a_t in enumerate((a1, a2)):
            s0_sl = s0p[:, br * 512 : br * 512 + N]
            for i in range(3):
                mms.append(nc.tensor.matmul(
                    s0_sl,
                    lhsT=WE[:, i * 128 : (i + 1) * 128],
                    rhs=shifted(a_t, i),
                    start=(i == 0),
                    stop=(i == 2),
                ))
        for br, (a_t, l_t) in enumerate(((a1, L1), (a2, L2))):
            s1_t = psum.tile([128, N], F32, tag=f"s1_{br}", name=f"s1_{br}")
            for i in range(3):
                mms.append(nc.tensor.matmul(
                    s1_t[:],
                    lhsT=WE[:, i * 128 : (i + 1) * 128],
                    rhs=shifted(l_t, i),
                    start=(i == 0),
                    stop=False,
                ))
            for i in range(3):
                mms.append(nc.tensor.matmul(
                    s1_t[:],
                    lhsT=WF[:, i * 128 : (i + 1) * 128],
                    rhs=shifted(a_t, i),
                    start=False,
                    stop=(i == 2),
                ))
            s1.append(s1_t)

        # ---- epilogue: y = exp(S1 * exp(-ln S0)); u/r per branch on ACT
        # (running under the S1 matmuls); the t/y/combine/bias/DMA tail is
        # split into 2 N-chunks so chunk 0 drains while chunk 1 computes ----
        s0v = s0p[:].rearrange("p (u v) -> p u v", u=2)[:, :, 0:N]
        u_t = pool.tile([128, 2 * N], F32)
        uv = u_t[:].rearrange("p (u v) -> p u v", u=2)
        u_inst = nc.scalar.activation(uv, s0v, AFT.Ln)
        r_t = pool.tile([128, 2 * N], F32)
        r_inst = nc.scalar.activation(r_t[:], u_t[:], AFT.Exp, scale=-1.0)
        ur_insts = [u_inst, r_inst]
        rts = [r_t[:, 0:N], r_t[:, N : 2 * N]]

        CL = N // 2
        t_insts, y_insts, bc_insts = [], [], []
        for ch in range(2):
            sl = slice(ch * CL, (ch + 1) * CL)
            ys = []
            for br in range(2):
                t_t = pool.tile([128, CL], F32, name=f"t_{br}_{ch}")
                t_insts.append(nc.vector.tensor_mul(
                    t_t[:], s1[br][:, sl], rts[br][:, sl]))
                y_t = pool.tile([128, CL], F32, name=f"y_{br}_{ch}")
                y_insts.append(nc.scalar.activation(y_t[:], t_t[:], AFT.Exp))
                ys.append(y_t)
            # combine on the PE: +-identity signs matmuls into PSUM
            if ch == 0:
                out_ps = psum.tile([O, 512], F32, tag="warm_ps",
                                   name=f"out_ps_{ch}")
            else:
                out_ps = psum.tile([O, 512], F32, name=f"out_ps_{ch}")
            mms.append(nc.tensor.matmul(
                out_ps[:, 0:CL], lhsT=signst[:, 0:O], rhs=ys[0][:],
                start=True, stop=False,
            ))
            mms.append(nc.tensor.matmul(
                out_ps[:, 0:CL], lhsT=signst[:, O : 2 * O], rhs=ys[1][:],
                start=False, stop=True,
            ))
            out_sb = pool.tile([O, CL], F32, name=f"out_sb_{ch}")
            bc_insts.append(nc.vector.tensor_scalar_add(
                out_sb[:], out_ps[:, 0:CL], biast[:]))
            nc.sync.dma_start(y_ap[:, sl], out_sb[:])
        _chain(mms, "PE order")

        # pin per-engine scheduling order along the dataflow
        _chain([aw_inst, we_inst, ln1_inst, ln2_inst] + ur_insts + y_insts,
               "ACT order")
        _chain([a1_inst, a2_inst, wf_inst, l1_inst, l2_inst] + t_insts
               + bc_insts, "DVE order")

    split_excess_waits(nc)
    return nc


_nc_cache = None


def _get_nc():
    global _nc_cache
    if _nc_cache is None:
        _nc_cache = build_nc()
    return _nc_cache


def _host_inputs(x, k1, k2, bias):
    """Build the 8 per-core input maps (pure layout, no arithmetic)."""
    # k [3,3,C,O] -> p=(i*3+j)*C+c -> [3(i), 96, O]; stack k1,k2 in M chunks
    k1f = np.ascontiguousarray(k1, np.float32).reshape(3, PS, O)
    k2f = np.ascontiguousarray(k2, np.float32).reshape(3, PS, O)
    k12 = np.concatenate([k1f, k2f], axis=2)  # [3, 96, 128]
    k12_sb = np.ascontiguousarray(k12.transpose(1, 0, 2).reshape(PS, 3 * 128))
    bias_sb = np.ascontiguousarray(bias, np.float32).reshape(O, 1)
    eye = np.eye(O, dtype=np.float32)
    signs = np.concatenate(
        [np.concatenate([eye, -eye], axis=0), np.concatenate([-eye, eye], axis=0)],
        axis=1,
    )  # [128, 128]: [:, 0:64] = [+I;-I] for y1, [:, 64:128] = [-I;+I] for y2

    in_maps = []
    for core in range(8):
        b, half = divmod(core, 2)
        h0 = half * HHALF
        xrow = np.ones((C, XPAD), np.float32)
        xrow[:, 0 : XROWS * W] = x[b, :, h0 : h0 + XROWS, :].reshape(
            C, XROWS * W
        )
        xs = np.empty((PS, REPW), np.float32)
        for j in range(3):
            xs[j * C : (j + 1) * C, :] = xrow[:, j : j + REPW]
        in_maps.append(
            {"x": xs, "k12": k12_sb, "bias": bias_sb, "signs": signs}
        )
    return in_maps


def kernel(x, k1, k2, bias):
    nc = _get_nc()
    in_maps = _host_inputs(x, k1, k2, bias)
    res = bass_utils.run_bass_kernel_spmd(
        nc, in_maps, core_ids=list(range(8)), trace=False
    )
    out = np.empty((B, O, HO, WO), np.float32)
    for core in range(8):
        b, half = divmod(core, 2)
        h0 = half * HHALF
        out[b, :, h0 : h0 + HHALF, :] = res.results[core]["y"].reshape(O, HHALF, WO)
    return out


if __name__ == "__main__":
    rng = np.random.default_rng(0)
    x = rng.standard_normal((B, C, H, W), dtype=np.float32)
    k1 = ((rng.random((KK, KK, C, O)) - 0.5) * 0.16).astype(np.float32)
    k2 = ((rng.random((KK, KK, C, O)) - 0.5) * 0.16).astype(np.float32)
    bias = np.zeros((O,), np.float32)
    out = kernel(x, k1, k2, bias)
    print("kernel out:", out.shape, out.dtype, float(np.abs(out).max()))
